# revision 28
# baseline (speedup 1.0000x reference)
"""Trainium2 Bass kernel for BasicConvClassifierWithSubject.

Pure data parallel over 8 cores (128 items/core). Per item the network is a
chain of PE matmuls (convs as shifted matmuls accumulating in PSUM, BN folded
into weights), gelu on ScalarE, residuals as identity/skip matmuls or DVE
adds. Spatial attention + per-subject 1x1 conv fuse on the host into one
per-item [271,128] stationary matrix (moe routing done as a host gather).

Tile blocks are 512 columns wide so channel-chunk pairs sit at a 512-column
stride, letting the 256-channel stages (b2c2/b3c1/b3c2) and the front
chunk-0/1 matmul run as fp8 DoubleRow matmuls: one 282-column instruction
contracts two K=128 tiles at once (2x PE throughput). Power-of-2 weight
scales keep fp8 in range and are undone by the activation input scale.
Items process in pairs sharing one 2-bank PSUM tile; two pairs interleave.
"""

import os
import numpy as np
import ml_dtypes

import concourse.bass as bass
import concourse.tile as tile
from concourse import bacc, mybir
from concourse.bass_utils import run_bass_kernel_spmd

f32 = mybir.dt.float32
f32r = mybir.dt.float32r
bf16 = mybir.dt.bfloat16
fp8 = mybir.dt.float8e4
DRM = mybir.MatmulPerfMode.DoubleRow
AF = mybir.ActivationFunctionType
AX = mybir.AxisListType
ALU = mybir.AluOpType
NPF8 = ml_dtypes.float8_e4m3

B, C, T = 1024, 271, 281
H, H2, E, NCLS, NSUBJ = 128, 256, 16, 1854, 4
EPS = 1e-5
TP = 512          # padded block length (data at cols 1..281, zeros at 0,282,283)
N = 282           # matmul moving size
KC = 3            # K chunks for the fused front matmul (271 -> 128+128+15)
PAR = 16          # X/Mg input slots (double-buffered group of 8)
NPP = 4           # pair slots (bounded by the 4 rotating PSUM tiles)
SLK = 514         # slack cols so rearrange-slices stay in bounds

MGS = 1024.0      # front stationary fp8 scale
WSC = 32.0        # b2c2/b3c1 fp8 weight scale (b3c2 uses 1.0)

_CACHE = {}


def _build(n_items, n_cores):
    assert n_items % 4 == 0
    nc = bacc.Bacc("TRN2", target_bir_lowering=False, debug=False,
                   num_devices=n_cores)

    def din(name, shape, dt=f32r):
        return nc.dram_tensor(name, shape, dt, kind="ExternalInput").ap()

    X = din("X", [n_items, C, T], fp8)
    Mg = din("Mg", [n_items, 128, KC * 128], fp8)
    D = din("D", [128, n_items], f32)
    cbias = din("cbias", [128, 10], f32)
    w_b1c1 = din("w_b1c1", [128, 3 * 128], f32r)
    w_b1c2 = din("w_b1c2", [128, 3 * 128], f32r)
    w_b2c1 = din("w_b2c1", [128, 2 * 256 + 2 * 128], fp8)  # DR(t0,t1) h0,h1 + t2 h0,h1
    w_b2sk = din("w_b2sk", [128, 2 * 128], fp8)
    w_b2c2 = din("w_b2c2", [128, 6 * 256], fp8)   # (h*3+k)*256: [c0|c1] pairs
    w_b3c1 = din("w_b3c1", [128, 6 * 256], fp8)
    w_b3c2 = din("w_b3c2", [128, 6 * 256], fp8)
    ident_r = din("ident_r", [128, 128], f32r)
    w1 = din("w1", [128, 2 * 128])
    w1x = din("w1x", [128, 128])
    rhsx = din("rhsx", [128, n_items])
    w2t = din("w2t", [128, NCLS])
    b2row = din("b2row", [1, NCLS])
    ones1 = din("ones1", [1, n_items])
    out = nc.dram_tensor("out", [n_items, NCLS], f32, kind="ExternalOutput").ap()

    with tile.TileContext(nc) as tc:
        wpool = tc.alloc_tile_pool(name="w", bufs=1)
        apool = tc.alloc_tile_pool(name="a", bufs=1)
        pspool = tc.alloc_tile_pool(name="ps", bufs=1, space="PSUM")

        def wtile(ap_, name):
            t = wpool.tile(list(ap_.shape), ap_.dtype, tag=name, name=name)
            nc.sync.dma_start(t[:], ap_[:])
            return t

        tD = wtile(D, "D")
        tcb = wtile(cbias, "cbias")
        tw_b1c1 = wtile(w_b1c1, "w_b1c1")
        tw_b1c2 = wtile(w_b1c2, "w_b1c2")
        tident_r = wtile(ident_r, "ident_r")
        tw_b2c1 = wtile(w_b2c1, "w_b2c1")
        tw_b2sk = wtile(w_b2sk, "w_b2sk")
        tw_b2c2 = wtile(w_b2c2, "w_b2c2")
        tw_b3c1 = wtile(w_b3c1, "w_b3c1")
        tw_b3c2 = wtile(w_b3c2, "w_b3c2")

        def zset(ap_):
            nc.vector.memset(ap_.bitcast(f32) if ap_.dtype == f32r else ap_, 0.0)

        def zpad(t, nblk):
            for blk in range(nblk):
                zset(t[:, blk * TP:blk * TP + 1])
                zset(t[:, blk * TP + 282:blk * TP + 284])

        # per-item padded input / front tiles (xp/mg are DMA'd so they get
        # PAR=16 slots for one-group prefetch; h0p is compute-written and the
        # PE queue is in-order, so 8 slots suffice)
        xp, mg, h0p = [], [], []
        for par in range(PAR):
            t = apool.tile([128, KC * TP + 2], fp8, tag=f"xp{par}",
                           name=f"xp{par}")
            zpad(t, KC)
            zset(t[:, 2 * TP:3 * TP])     # chunk2 rows 15..127 stay zero
            xp.append(t)
            mg.append(apool.tile([128, KC * 128], fp8, tag=f"mg{par}",
                                 name=f"mg{par}"))
        for par in range(8):
            t0 = apool.tile([128, TP], f32r, tag=f"h0p{par}", name=f"h0p{par}")
            zset(t0[:, 0:1])
            zset(t0[:, 282:284])
            h0p.append(t0)

        # pair tiles: blocks of TP cols; 2-chunk tiles are sub-major:
        # block (sub, chunk) at col sub*2*TP + chunk*TP
        def pairt(name, chunks, dt):
            ts_ = []
            for pp in range(NPP):
                t = apool.tile([128, chunks * 2 * TP + SLK], dt,
                               tag=f"{name}{pp}", name=f"{name}{pp}")
                zpad(t, chunks * 2)
                ts_.append(t)
            return ts_

        y1p = pairt("y1p", 1, f32r)
        # h1p blocks per sub: [orig | dup shifted -1] so DR tap pairs sit at
        # the proven 512-col k-tile stride
        h1p = pairt("h1p", 2, fp8)
        y2p = pairt("y2p", 2, fp8)
        h2p = pairt("h2p", 2, fp8)
        y3p = pairt("y3p", 2, fp8)
        h3p = pairt("h3p", 2, f32)
        tmp3 = [apool.tile([128, 2 * N], f32, tag=f"tmp3_{i}", name=f"tmp3_{i}")
                for i in range(8)]

        V = [apool.tile([128, n_items], f32, tag=f"V{c}", name=f"V{c}")
             for c in range(2)]

        psum_ctr = [0]

        def pwide():
            t = pspool.tile([128, 1024], f32, tag=f"ps{psum_ctr[0] % 4}",
                            name=f"psum{psum_ctr[0]}")
            psum_ctr[0] += 1
            return t

        def blkwin(tl, blk, k):
            return tl[:, blk * TP + k: blk * TP + k + N]

        def drmov(tl, base):
            """[128, 2, N] DoubleRow moving view: k-tiles at base, base+TP."""
            return (tl[:, base:base + 2 * TP]
                    .rearrange("p (i c) -> p i c", i=2)[:, :, 0:N])

        def drw(tl, slot):
            """[128, 2, 128] DoubleRow stationary view at col slot*256."""
            return (tl[:, slot * 256:slot * 256 + 256]
                    .rearrange("p (i m) -> p i m", i=2))

        def pair_act(p, dst, bias_col, scale=1.0):
            """1-chunk tiles: psum {0,512}+0..280 -> dst blocks cols 1..281."""
            pin = p[:].rearrange("m (i c) -> m i c", i=2)[:, :, 0:T]
            sout = (dst[:, 0:2 * TP]
                    .rearrange("m (i c) -> m i c", i=2)[:, :, 1:1 + T])
            nc.scalar.activation(sout, pin, AF.Gelu,
                                 bias=tcb[:, bias_col:bias_col + 1],
                                 scale=scale)

        def pair_act2(p, dst, ch, bias_col, scale=1.0):
            """2-chunk sub-major tiles: write chunk ch of both subs."""
            pin = p[:].rearrange("m (i c) -> m i c", i=2)[:, :, 0:T]
            sout = (dst[:, ch * TP:ch * TP + 2 * 2 * TP]
                    .rearrange("m (i c) -> m i c", i=2)[:, :, 1:1 + T])
            nc.scalar.activation(sout, pin, AF.Gelu,
                                 bias=tcb[:, bias_col:bias_col + 1],
                                 scale=scale)

        def st_load(b, par):
            xt = xp[par]
            for ch in range(KC):
                rows = 128 if ch < 2 else C - 2 * 128
                nc.sync.dma_start(
                    xt[:rows, ch * TP + 1: ch * TP + 1 + T],
                    X[b, ch * 128: ch * 128 + rows, :])
            nc.sync.dma_start(mg[par][:], Mg[b])

        def st_front(b0, pars, pp):
            p = pwide()
            for sub in range(2):
                par = pars[sub]
                reg = p[:, sub * 512: sub * 512 + N]
                nc.tensor.matmul(reg, drw(mg[par], 0), drmov(xp[par], 1),
                                 start=True, stop=False, perf_mode=DRM)
                nc.tensor.matmul(reg, mg[par][:, 256:384],
                                 blkwin(xp[par], 2, 1), start=False, stop=True)
            for sub in range(2):
                nc.vector.tensor_scalar(
                    h0p[pars[sub] % 8][:, 1:1 + T],
                    p[:, sub * 512: sub * 512 + T],
                    1.0 / MGS, tD[:, b0 + sub:b0 + sub + 1],
                    ALU.mult, ALU.add)

        def st_b1c1(b0, pars, pp):
            p = pwide()
            for sub in range(2):
                for k in range(3):
                    nc.tensor.matmul(p[:, sub * 512: sub * 512 + N],
                                     tw_b1c1[:, k * 128:(k + 1) * 128],
                                     blkwin(h0p[pars[sub] % 8], 0, k),
                                     start=(k == 0), stop=(k == 2))
            pair_act(p, y1p[pp], 0)

        def st_b1c2(b0, pars, pp):
            p = pwide()
            for sub in range(2):
                reg = p[:, sub * 512: sub * 512 + N]
                for k in range(3):
                    nc.tensor.matmul(reg, tw_b1c2[:, k * 128:(k + 1) * 128],
                                     blkwin(y1p[pp], sub, k),
                                     start=(k == 0), stop=False)
                nc.tensor.matmul(reg, tident_r[:],
                                 blkwin(h0p[pars[sub] % 8], 0, 1),
                                 start=False, stop=True)
            pair_act2(p, h1p[pp], 0, 1)
            # dup[c] = orig[c+1] per sub, for the b2c1 DoubleRow tap pair
            ov = (h1p[pp][:, TP:TP + 2 * 2 * TP]
                  .rearrange("m (i c) -> m i c", i=2)[:, :, 0:N])
            iv = (h1p[pp][:, 0:2 * 2 * TP]
                  .rearrange("m (i c) -> m i c", i=2)[:, :, 1:1 + N])
            nc.vector.tensor_copy(ov, iv)

        def st_b2c1(b0, pars, pp, h):
            p = pwide()
            for sub in range(2):
                nc.tensor.matmul(p[:, sub * 512: sub * 512 + N],
                                 drw(tw_b2c1, h),
                                 drmov(h1p[pp], sub * 2 * TP),
                                 start=True, stop=False, perf_mode=DRM)
            for sub in range(2):
                nc.tensor.matmul(p[:, sub * 512: sub * 512 + N],
                                 tw_b2c1[:, 512 + h * 128:512 + (h + 1) * 128],
                                 h1p[pp][:, sub * 2 * TP + 2:sub * 2 * TP + 2 + N],
                                 start=False, stop=True)
            pair_act2(p, y2p[pp], h, 2 + h, scale=1.0 / WSC)

        def st_b2c2(b0, pars, pp, h):
            p = pwide()
            for k in range(3):
                for sub in range(2):
                    nc.tensor.matmul(p[:, sub * 512: sub * 512 + N],
                                     drw(tw_b2c2, h * 3 + k),
                                     drmov(y2p[pp], sub * 2 * TP + k),
                                     start=(k == 0), stop=False, perf_mode=DRM)
            for sub in range(2):
                nc.tensor.matmul(p[:, sub * 512: sub * 512 + N],
                                 tw_b2sk[:, h * 128:(h + 1) * 128],
                                 blkwin(h1p[pp], 2 * sub, 1),
                                 start=False, stop=True)
            pair_act2(p, h2p[pp], h, 4 + h, scale=1.0 / WSC)

        def st_b3c1(b0, pars, pp, h):
            p = pwide()
            for k in range(3):
                for sub in range(2):
                    nc.tensor.matmul(p[:, sub * 512: sub * 512 + N],
                                     drw(tw_b3c1, h * 3 + k),
                                     drmov(h2p[pp], sub * 2 * TP + k),
                                     start=(k == 0), stop=(k == 2),
                                     perf_mode=DRM)
            pair_act2(p, y3p[pp], h, 6 + h, scale=1.0 / WSC)

        def st_b3c2(b0, pars, pp, h):
            p = pwide()
            for k in range(3):
                for sub in range(2):
                    nc.tensor.matmul(p[:, sub * 512: sub * 512 + N],
                                     drw(tw_b3c2, h * 3 + k),
                                     drmov(y3p[pp], sub * 2 * TP + k),
                                     start=(k == 0), stop=(k == 2),
                                     perf_mode=DRM)
            # residual add on DVE (both items in one op), then pair gelu
            tm = tmp3[pp * 2 + h]
            pin = p[:].rearrange("m (i c) -> m i c", i=2)[:, :, 0:N]
            res = (h2p[pp][:, h * TP:h * TP + 2 * 2 * TP]
                   .rearrange("m (i c) -> m i c", i=2)[:, :, 1:1 + N])
            tview = tm[:].rearrange("m (i c) -> m i c", i=2)
            nc.vector.tensor_add(tview, pin, res)
            sout = (h3p[pp][:, h * TP:h * TP + 2 * 2 * TP]
                    .rearrange("m (i c) -> m i c", i=2)[:, :, 1:1 + T])
            nc.scalar.activation(sout, tview[:, :, 0:T], AF.Gelu,
                                 bias=tcb[:, 8 + h:9 + h])

        def st_pool(b, par, pp, sub):
            for c in range(2):
                nc.vector.reduce_sum(
                    V[c][:, b:b + 1],
                    h3p[pp][:, sub * 2 * TP + c * TP: sub * 2 * TP + c * TP + 284],
                    axis=AX.X)

        def pair_stages(pp):
            out_ = [st_front, st_b1c1, st_b1c2]
            outl = [lambda b0, pars, pp=pp, f=f: f(b0, pars, pp) for f in out_]
            for f in (st_b2c1, st_b2c2, st_b3c1, st_b3c2):
                for h in range(2):
                    outl.append(lambda b0, pars, pp=pp, f=f, h=h: f(b0, pars, pp, h))
            return outl

        # group of 8 items = 4 pairs, stage-interleaved between the pairs so
        # the PE always has independent work while acts drain other pairs
        def load_group(g0):
            for b in range(g0, min(g0 + 8, n_items)):
                st_load(b, b % PAR)

        load_group(0)
        for g0 in range(0, n_items, 8):
            bs = list(range(g0, g0 + 8))
            pars = [b % PAR for b in bs]
            pps = [(g0 // 2 + j) % NPP for j in range(4)]
            if g0 + 8 < n_items:
                load_group(g0 + 8)
            sts = [pair_stages(pps[j]) for j in range(4)]
            for row in zip(*sts):
                for j, sfn in enumerate(row):
                    sfn(bs[2 * j], pars[2 * j:2 * j + 2])
            for i in range(8):
                st_pool(bs[i], pars[i], pps[i // 2], i % 2)

        # ---- head ----
        tw1 = wtile(w1, "w1")
        tw1x = wtile(w1x, "w1x")
        trhsx = wtile(rhsx, "rhsx")
        tw2t = wtile(w2t, "w2t")
        tb2row = wtile(b2row, "b2row")
        tones1 = wtile(ones1, "ones1")

        Vr = [apool.tile([128, n_items], f32r, tag=f"Vr{c}", name=f"Vr{c}")
              for c in range(2)]
        for c in range(2):
            nc.vector.tensor_copy(Vr[c][:], V[c][:])

        ph = pwide()[:, :n_items]
        for c in range(2):
            nc.tensor.matmul(ph[:], tw1[:, c * 128:(c + 1) * 128], Vr[c][:],
                             start=(c == 0), stop=False)
        nc.tensor.matmul(ph[:], tw1x[:], trhsx[:], start=False, stop=True)
        hmid = apool.tile([128, n_items], f32r, tag="hmid", name="hmid")
        nc.scalar.activation(hmid[:], ph[:], AF.Relu)

        out_sb = apool.tile([n_items, NCLS], f32, tag="out_sb", name="out_sb")
        nsplit = [512, 512, 512, NCLS - 3 * 512]
        off = 0
        for w_ in nsplit:
            po = pwide()[:n_items, :w_]
            nc.tensor.matmul(po[:], hmid[:], tw2t[:, off:off + w_],
                             start=True, stop=False)
            nc.tensor.matmul(po[:], tones1[:], tb2row[:, off:off + w_],
                             start=False, stop=True)
            nc.vector.tensor_copy(out_sb[:, off:off + w_], po[:])
            off += w_
        nc.sync.dma_start(out[:, :], out_sb[:, :])

        for p_ in (pspool, apool, wpool):
            p_.release()

    nc.compile()
    return nc


def _preprocess(inputs):
    f = np.float64

    attn = inputs["attention"].astype(f)
    attn = attn - attn.max(axis=1, keepdims=True)
    np.exp(attn, out=attn)
    attn /= attn.sum(axis=1, keepdims=True)
    A = inputs["sa_w"].astype(f) @ attn
    subj_w = inputs["subj_w"].astype(f)
    M = np.einsum("shk,kc->shc", subj_w, A)
    MT = np.zeros((NSUBJ, KC * 128, H), np.float32)
    MT[:, :C, :] = np.transpose(M, (0, 2, 1))
    MT = (MT.reshape(NSUBJ, KC, 128, H).transpose(0, 2, 1, 3)
            .reshape(NSUBJ, 128, KC * 128) * MGS).astype(NPF8)
    Dall = (np.einsum("shk,k->sh", subj_w, inputs["sa_b"].astype(f))
            + inputs["subj_b"].astype(f)).astype(np.float32)

    inv = 1.0 / np.sqrt(1.0 + EPS)

    def fold(w, b, g, be):
        s = g.astype(f) * inv
        wf = w.astype(f) * s[:, None, None]
        bf_ = s * b.astype(f) + be.astype(f)
        return wf, bf_.astype(np.float32)

    def pack_taps(wf, cin_chunks, cout_halves):
        blocks = []
        for k in range(3):
            for c in range(cin_chunks):
                for h in range(cout_halves):
                    blk = wf[h * 128:(h + 1) * 128, c * 128:(c + 1) * 128, k].T
                    blocks.append(blk)
        return np.concatenate(blocks, axis=1).astype(np.float32)

    def pack_dr(wf, sc):
        # (h*3+k)*256 blocks: [w_{h,c0,k}.T | w_{h,c1,k}.T] * sc, fp8
        blocks = []
        for h in range(2):
            for k in range(3):
                for c in range(2):
                    blocks.append(
                        wf[h * 128:(h + 1) * 128, c * 128:(c + 1) * 128, k].T)
        return (np.concatenate(blocks, axis=1) * sc).astype(NPF8)

    w11, b11 = fold(inputs["b1_c1w"], inputs["b1_c1b"], inputs["b1_g1"], inputs["b1_be1"])
    w12, b12 = fold(inputs["b1_c2w"], inputs["b1_c2b"], inputs["b1_g2"], inputs["b1_be2"])
    w21, b21 = fold(inputs["b2_c1w"], inputs["b2_c1b"], inputs["b2_g1"], inputs["b2_be1"])
    w22, b22 = fold(inputs["b2_c2w"], inputs["b2_c2b"], inputs["b2_g2"], inputs["b2_be2"])
    w31, b31 = fold(inputs["b3_c1w"], inputs["b3_c1b"], inputs["b3_g1"], inputs["b3_be1"])
    w32, b32 = fold(inputs["b3_c2w"], inputs["b3_c2b"], inputs["b3_g2"], inputs["b3_be2"])
    skw = inputs["b2_skw"][:, :, 0].astype(np.float64)
    skb = inputs["b2_skb"].astype(np.float32)
    b22 = b22 + skb

    cbias = np.zeros((128, 10), np.float32)
    cbias[:, 0] = b11
    cbias[:, 1] = b12
    cbias[:, 2], cbias[:, 3] = b21[:128], b21[128:]
    cbias[:, 4], cbias[:, 5] = b22[:128], b22[128:]
    cbias[:, 6], cbias[:, 7] = b31[:128], b31[128:]
    cbias[:, 8], cbias[:, 9] = b32[:128], b32[128:]

    head_w1 = inputs["head_w1"].astype(f)
    w1pack = np.concatenate(
        [(head_w1[:, c * 128:(c + 1) * 128] / T).T.astype(np.float32) for c in range(2)],
        axis=1)
    w1x = np.zeros((128, 128), np.float32)
    w1x[:E, :] = head_w1[:, 2 * 128:2 * 128 + E].T
    w1x[E, :] = inputs["head_b1"]
    w2t = inputs["head_w2"].T.astype(np.float32)
    b2row = inputs["head_b2"].astype(np.float32)[None, :]

    shared = dict(
        cbias=cbias,
        w_b1c1=pack_taps(w11, 1, 1),
        w_b1c2=pack_taps(w12, 1, 1),
        w_b2c1=(np.concatenate(
            [w21[0:128, :, 0].T, w21[0:128, :, 1].T,
             w21[128:256, :, 0].T, w21[128:256, :, 1].T,
             w21[0:128, :, 2].T, w21[128:256, :, 2].T],
            axis=1) * WSC).astype(NPF8),
        w_b2sk=(np.concatenate([skw[:128].T, skw[128:].T], axis=1)
                * WSC).astype(NPF8),
        w_b2c2=pack_dr(w22, WSC),
        w_b3c1=pack_dr(w31, WSC),
        w_b3c2=pack_dr(w32, 1.0),
        ident_r=np.eye(128, dtype=np.float32),
        w1=w1pack, w1x=w1x, w2t=w2t, b2row=b2row,
    )

    sidx = inputs["subject_idxs"].astype(np.int64)
    Mg = MT[sidx]
    Dcols = Dall[sidx].T.astype(np.float32)
    emb = inputs["emb"].astype(np.float32)
    embG = emb[sidx].T
    return shared, Mg, Dcols, embG


def _run(inputs, n_items, n_cores):
    key = (n_items, n_cores)
    if key not in _CACHE:
        _CACHE[key] = _build(n_items, n_cores)
    nc = _CACHE[key]

    shared, Mg, Dcols, embG = _preprocess(inputs)
    X = np.ascontiguousarray(inputs["X"], dtype=NPF8)

    in_maps = []
    for c in range(n_cores):
        lo, hi = c * n_items, (c + 1) * n_items
        rhsx = np.zeros((128, n_items), np.float32)
        rhsx[:E, :] = embG[:, lo:hi]
        rhsx[E, :] = 1.0
        m = dict(shared)
        m["X"] = X[lo:hi]
        m["Mg"] = np.ascontiguousarray(Mg[lo:hi])
        m["D"] = np.ascontiguousarray(Dcols[:, lo:hi])
        m["rhsx"] = rhsx
        m["ones1"] = np.ones((1, n_items), np.float32)
        in_maps.append(m)

    trace = bool(int(os.environ.get("KTRACE", "0")))
    if trace:
        try:
            from antenv.axon_hooks import (get_axon_ntff_profile_hook,
                                           set_axon_ntff_profile_hook)
            if get_axon_ntff_profile_hook() is None:
                from trn_agent_boot.trn_boot import _ntff_profile_via_ctypes
                set_axon_ntff_profile_hook(
                    _ntff_profile_via_ctypes("/opt/axon/libaxon_pjrt.so"))
        except Exception as e:
            print(f"(ntff hook unavailable: {e})")
    res = run_bass_kernel_spmd(nc, in_maps, core_ids=list(range(n_cores)),
                               trace=trace)
    outp = np.concatenate([res.results[c]["out"] for c in range(n_cores)], axis=0)
    if trace:
        print(f"HW exec time: {res.exec_time_ns} ns "
              f"(mean {res.mean_exec_time_ns}, max core {res.max_exec_time_core_id})")
    return outp, res


def kernel(**inputs):
    outp, _ = _run(inputs, B // 8, 8)
    return outp
